# revision 19
# baseline (speedup 1.0000x reference)
"""Trainium2 Bass kernel for nn_Decoder_TRANSFORMER_14791867367496.

The reference decoder is affine in the positions: each frame step is
    pos_{t+1} = pos_t @ M + (d_t[b] + g[b,j]),   M = I + W_pe @ W3  (3x3)
(with W_final = [W1; W2; W3] split along its 768 input rows), so the whole
60-step scan has a closed form

    out[b, j, :, t] = X[b, j, :] @ Q_t + r_t[b, :]

where X = initial_grid,
    Q_t = M^t + (W_pe @ W2) @ S_t,          S_t = sum_{k<t} M^k
    r_t[b] = h @ S_t + D_t[b],              D_t = sum_{s=1..t} d_s M^{t-s}
    d_t[b] = (emb_table[t] + z @ W_clip + b_clip) @ W1
    h      = b_pe @ (W2 + W3) + b_final

All of Q/r are tiny (3x3 / per-batch 3-vectors) and are computed on the host
in float64.  The device kernel is then a single affine map per point
([3 feats + bias] -> 180 outputs) and is purely output-bandwidth bound
(94 MB of f32 output).

Precision trick: fp32 matmuls on trn2 run as two PE passes (~2x slower
streaming + 2x LDWEIGHTS).  Instead each operand is split into three bf16
chunks (x = x0+x1+x2, 8 mantissa bits each) and all cross terms with
a+b <= 2 are summed IN A SINGLE MATMUL by stacking them along the
contraction dim: rows [x0 x0 x1 x0 x1 x2] paired against
[q0 q1 q0 q2 q1 q0].  bf16 products are exact in fp32, so this matches
fp32 accuracy (~3e-8 measured) at bf16 speed.  Per point-pair-tile the
K-stack is 21 rows x 2 tiles = K=42, N=2*180=360 (block-diagonal rhs).

Sharding: data-parallel over batch — each of the 8 cores handles 4 batches
(16384 points = 128 point-tiles = 64 packed matmuls).  Output streams out
in fully-linear ~1.47 MB DMAs (the first group goes out as eighth/
eighth/quarter/half so the output stream starts right after matmul 0).  The device program is
raw Bacc (no Tile framework) with hand-rolled per-edge semaphores: PE runs
the matmuls, DVE/ACT alternate PSUM->SBUF copies within every group, and
the two HWDGE rings (SP + ACT) split the input-chunk issue load so ACT is
free to copy from group 0 on.  The output stream is SDMA-saturated end to
end: measured ~41 us max-core (~37 us mean) against a ~35 us HBM-write
floor for the 94 MB output.
"""

import numpy as np

BS, NFRAMES, NJOINTS, NFEATS, LATENT, CLIP = 32, 60, 4096, 3, 256, 512
NCORES = 8
B_PER_CORE = BS // NCORES                  # 4
PTS = B_PER_CORE * NJOINTS                 # 16384 points per core
NTILES = PTS // 128                        # 128 point-tiles per core
GROUPS = 8                                 # output DMA groups
TPG = NTILES // GROUPS                     # 16 tiles per group
FC = NFEATS * NFRAMES                      # 180 output columns per point
KR = 21                                    # K-stack rows per tile (3*6 + 3 bias)
PAIR = 2                                   # tiles fused per matmul
MM_PER_G = TPG // PAIR                     # 8 matmuls per group
XCH = [0, 0, 1, 0, 1, 2]                   # x-chunk index per K row
QCH = [0, 1, 0, 2, 1, 0]                   # q-chunk index per K row


def _split3(a):
    """Split f32 array into three bf16 chunks whose sum reproduces ~24
    mantissa bits.  Returned as f32 arrays holding bf16-representable
    values."""
    import ml_dtypes
    bf = ml_dtypes.bfloat16
    a = np.asarray(a, np.float32)
    a0 = a.astype(bf).astype(np.float32)
    a1 = (a - a0).astype(bf).astype(np.float32)
    a2 = (a - a0 - a1).astype(bf).astype(np.float32)
    return a0, a1, a2


def _precompute(z, W_pe, b_pe, W_clip, b_clip, emb_table, W_final, b_final):
    """Host-side f64 computation of the closed-form coefficients.

    Returns Q_all [3, 180] and r_all [32, 180], column layout c = f*60 + t
    (matching the [.., 3, 60] innermost layout of the output)."""
    f64 = np.float64
    W_pe64 = np.asarray(W_pe, f64)
    W_fin = np.asarray(W_final, f64)
    W1, W2, W3 = W_fin[:LATENT], W_fin[LATENT:2 * LATENT], W_fin[2 * LATENT:]
    M = np.eye(3) + W_pe64 @ W3
    Gm = W_pe64 @ W2
    b_pe64 = np.asarray(b_pe, f64)
    h = b_pe64 @ W2 + b_pe64 @ W3 + np.asarray(b_final, f64)
    z_proj = np.asarray(z, f64) @ np.asarray(W_clip, f64) + np.asarray(b_clip, f64)
    d = (np.asarray(emb_table, f64)[None, :, :] + z_proj[:, None, :]) @ W1  # [32,60,3]

    Q = np.zeros((NFRAMES, 3, 3))
    R = np.zeros((NFRAMES, BS, 3))
    Q[0] = np.eye(3)
    Mt = np.eye(3)
    S = np.zeros((3, 3))
    D = np.zeros((BS, 3))
    for t in range(1, NFRAMES):
        S = S + Mt
        Mt = Mt @ M
        D = D @ M + d[:, t, :]
        Q[t] = Mt + Gm @ S
        R[t] = h @ S + D
    Q_all = Q.transpose(1, 2, 0).reshape(3, FC)     # [k, f*60+t]
    r_all = R.transpose(1, 2, 0).reshape(BS, FC)    # [b, f*60+t]
    return Q_all.astype(np.float32), r_all.astype(np.float32)


N_PS = 8      # psum slots (one bank each; a group cycles all 8)
N_STAGE = 3   # stage buffers


def _copy_seq(j):
    """(engine, 1-based position of copy j within that engine's stream).

    Copies alternate DVE/ACT by matmul index so both engines share every
    group's copy wall."""
    return ("v" if j % 2 == 0 else "a"), j // 2 + 1


def _build_bass():
    import concourse.mybir as mybir
    from concourse import bacc
    from concourse.bass import ts

    f32 = mybir.dt.float32
    bf16 = mybir.dt.bfloat16
    nc = bacc.Bacc(None, target_bir_lowering=False)
    xt = nc.dram_tensor("xt", [PAIR * KR, NTILES // PAIR * 128], bf16,
                        kind="ExternalInput")
    rhs = nc.dram_tensor("rhs", [PAIR * KR, B_PER_CORE * PAIR * FC], bf16,
                         kind="ExternalInput")
    out = nc.dram_tensor("out", [PTS, FC], f32, kind="ExternalOutput")
    out_v = out[:].rearrange("(g j w) c -> g j (w c)", g=GROUPS, j=128, w=TPG)

    from contextlib import ExitStack
    ctx = ExitStack()
    rhs_sb = ctx.enter_context(
        nc.sbuf_tensor("rhs_sb", [PAIR * KR, B_PER_CORE * PAIR * FC], bf16))
    xt_sb = [ctx.enter_context(
        nc.sbuf_tensor(f"xt_sb{g}", [PAIR * KR, MM_PER_G * 128], bf16))
        for g in range(GROUPS)]
    stage = [ctx.enter_context(
        nc.sbuf_tensor(f"stage{i}", [128, TPG * FC], f32))
        for i in range(N_STAGE)]
    psum = [ctx.enter_context(
        nc.psum_tensor(f"psum{i}", [128, PAIR * FC], f32))
        for i in range(N_PS)]
    s_rhs = ctx.enter_context(nc.semaphore("s_rhs"))
    s_c0a = ctx.enter_context(nc.semaphore("s_c0a"))
    s_chunk = [ctx.enter_context(nc.semaphore(f"s_chunk{g}"))
               for g in range(GROUPS)]
    s_pe = ctx.enter_context(nc.semaphore("s_pe"))
    s_cpv = ctx.enter_context(nc.semaphore("s_cpv"))
    s_cpa = ctx.enter_context(nc.semaphore("s_cpa"))
    s_slot = [ctx.enter_context(nc.semaphore(f"s_slot{i}"))
              for i in range(N_STAGE)]

    # ---- input DMAs ----
    # chunk0a and rhs are issued simultaneously on the two HWDGE rings so
    # the first matmul's operands land ASAP; remaining chunks are split so
    # each ring's issue backlog clears before it is needed for real work.
    half = MM_PER_G * 128 // 2
    nc.scalar.dma_start(out=xt_sb[0][:, :half],
                        in_=xt[:, :half]).then_inc(s_c0a, 16)
    nc.sync.dma_start(out=rhs_sb[:], in_=rhs[:]).then_inc(s_rhs, 16)
    nc.scalar.dma_start(out=xt_sb[0][:, half:],
                        in_=xt[:, half:MM_PER_G * 128]).then_inc(s_chunk[0], 16)
    nc.sync.dma_start(
        out=xt_sb[1][:], in_=xt[:, ts(1, MM_PER_G * 128)]
    ).then_inc(s_chunk[1], 16)
    for g in range(2, GROUPS):
        eng = nc.scalar if g in (2, 3) else nc.sync
        eng.dma_start(
            out=xt_sb[g][:], in_=xt[:, ts(g, MM_PER_G * 128)]
        ).then_inc(s_chunk[g], 16)

    # out-DMA inc totals per stage slot, recorded in emission order so the
    # slot-reuse waits below match however many DMAs read the slot.
    slot_incs = [0] * N_STAGE

    def copies(engine, s_cp_self, g, parity):
        st = stage[g % N_STAGE]
        for sp in range(parity, MM_PER_G, 2):
            j = g * MM_PER_G + sp
            if sp == parity and g >= N_STAGE:
                # stage slot reuse: wait for every out-DMA that read it
                engine.wait_ge(s_slot[g % N_STAGE],
                               16 * slot_reads_before[g])
            engine.wait_ge(s_pe, j + 1)
            if parity == 0:
                nc.vector.tensor_copy(
                    out=st[:, ts(sp, PAIR * FC)], in_=psum[j % N_PS][:]
                ).then_inc(s_cp_self, 1)
            else:
                nc.scalar.copy(
                    out=st[:, ts(sp, PAIR * FC)], in_=psum[j % N_PS][:]
                ).then_inc(s_cp_self, 1)

    # number of completed out-DMA incs required on slot g%3 before group g
    # may overwrite it (group 0's stage goes out as two half-DMAs)
    dma_count = {0: 4}
    slot_reads_before = {}
    seen = [0] * N_STAGE
    for g in range(GROUPS):
        slot_reads_before[g] = seen[g % N_STAGE]
        seen[g % N_STAGE] += dma_count.get(g, 1)

    for g in range(GROUPS):
        copies(nc.scalar, s_cpa, g, 1)

    # ---- DVE: even-slot copies ----
    for g in range(GROUPS):
        copies(nc.vector, s_cpv, g, 0)

    # ---- PE: matmuls ----
    for g in range(GROUPS):
        lb = g // 2
        for sp in range(MM_PER_G):
            j = g * MM_PER_G + sp
            if g == 0:
                if sp == 0:
                    nc.tensor.wait_ge(s_c0a, 16)
                    nc.tensor.wait_ge(s_rhs, 16)
                elif sp == MM_PER_G // 2:
                    nc.tensor.wait_ge(s_chunk[0], 16)
            elif sp == 0:
                nc.tensor.wait_ge(s_chunk[g], 16)
            if j >= N_PS:
                # psum slot reuse: wait for the copy that drained it
                eng, pos = _copy_seq(j - N_PS)
                nc.tensor.wait_ge(s_cpv if eng == "v" else s_cpa, pos)
            nc.tensor.matmul(
                psum[j % N_PS][:],
                xt_sb[g][:, ts(sp, 128)],
                rhs_sb[:, ts(lb, PAIR * FC)],
                start=True, stop=True,
            ).then_inc(s_pe, 1)

    # ---- SP: output DMAs ----
    out_v4 = out[:].rearrange("(g j w) c -> g j w c", g=GROUPS, j=128, w=TPG)
    for g in range(GROUPS):
        if g == 0:
            # eighth/eighth/quarter/half DMAs: the stream starts right
            # after matmul 0's copy lands
            for nv, na, w0, w1 in ((1, 0, 0, 2), (1, 1, 2, 4),
                                   (2, 2, 4, 8), (4, 4, 8, TPG)):
                nc.sync.wait_ge(s_cpv, nv)
                if na:
                    nc.sync.wait_ge(s_cpa, na)
                nc.sync.dma_start(
                    out=out_v4[0][:, w0:w1, :],
                    in_=stage[0][:, w0 * FC:w1 * FC],
                ).then_inc(s_slot[0], 16)
            continue
        n_half = MM_PER_G * (g + 1) // 2
        nc.sync.wait_ge(s_cpv, n_half)
        nc.sync.wait_ge(s_cpa, n_half)
        nc.sync.dma_start(out=out_v[g], in_=stage[g % N_STAGE][:]).then_inc(
            s_slot[g % N_STAGE], 16)

    ctx.close()
    nc.finalize()
    return nc


_NC_CACHE = None
_LAST_RESULTS = None  # BassKernelResults of the most recent run (for profiling)


def kernel(z, mask, initial_grid, W_pe, b_pe, W_clip, b_clip, emb_table,
           W_final, b_final):
    global _NC_CACHE, _LAST_RESULTS
    import ml_dtypes
    from concourse import bass_utils

    bf = ml_dtypes.bfloat16
    Q_all, r_all = _precompute(z, W_pe, b_pe, W_clip, b_clip, emb_table,
                               W_final, b_final)
    Qs = _split3(Q_all)                                 # 3 x [3, 180]
    X = np.ascontiguousarray(np.asarray(initial_grid), dtype=np.float32)

    in_maps = []
    for c in range(NCORES):
        Xc = X[B_PER_CORE * c:B_PER_CORE * (c + 1)].reshape(PTS, NFEATS)
        # point p = g*2048 + j*16 + w lives at tile (g, w), psum partition j
        X4 = Xc.reshape(GROUPS, 128, TPG, NFEATS).transpose(3, 0, 2, 1)
        ch = _split3(X4)                                # 3 x [3, 8, 16, 128]
        A = np.empty((GROUPS, TPG, KR, 128), np.float32)
        for k in range(NFEATS):
            for m in range(6):
                A[:, :, 6 * k + m, :] = ch[XCH[m]][k]
        A[:, :, 18:21, :] = 1.0                         # bias rows
        # matmul s covers tiles (2*(s%8), 2*(s%8)+1) of group s//8;
        # stationary rows 21a.. hold tile a of the pair
        xt_host = (A.reshape(GROUPS, MM_PER_G, PAIR, KR, 128)
                   .transpose(2, 3, 0, 1, 4)
                   .reshape(PAIR * KR, NTILES // PAIR * 128)).astype(bf)

        rhs_host = np.zeros((PAIR * KR, B_PER_CORE * PAIR * FC), np.float32)
        for lb in range(B_PER_CORE):
            rs = _split3(r_all[B_PER_CORE * c + lb])    # 3 x [180]
            R = np.empty((KR, FC), np.float32)
            for k in range(NFEATS):
                for m in range(6):
                    R[6 * k + m] = Qs[QCH[m]][k]
            R[18:21] = np.stack(rs)
            for a in range(PAIR):                       # block-diagonal
                rhs_host[KR * a:KR * (a + 1),
                         lb * PAIR * FC + FC * a: lb * PAIR * FC + FC * (a + 1)] = R
        in_maps.append({"xt": np.ascontiguousarray(xt_host),
                        "rhs": rhs_host.astype(bf)})

    if _NC_CACHE is None:
        _NC_CACHE = _build_bass()
    res = bass_utils.run_bass_kernel_spmd(
        _NC_CACHE, in_maps, core_ids=list(range(NCORES))
    )
    _LAST_RESULTS = res

    out = np.empty((BS, NJOINTS, NFEATS, NFRAMES), np.float32)
    for c in range(NCORES):
        out[B_PER_CORE * c:B_PER_CORE * (c + 1)] = (
            res.results[c]["out"].reshape(B_PER_CORE, NJOINTS, NFEATS, NFRAMES)
        )
    return out
